# revision 2
# baseline (speedup 1.0000x reference)
"""Trainium2 Bass kernel for nn_MDCN (mixture-density head forward), v3.

Reference (B=2048, F=1024, M=128):
    rho = tanh(feature @ h2rho_w.T + h2rho_b);  rho[:, 0] = 0.95
    pi  = softmax(feature @ h2pi_w.T + h2pi_b)
    var0 = exp(feature @ h2var_w.T + h2var_b)
    var = (1 - exp(rho)) * var0 + 1e-4
    W_ = r*muW + s*(r*(zstd/wstd)*(W-muW) + Z*s),  s = sqrt(1-r^2)
    mu = einsum('bmf,bf->bm', W_, feature)

Algebra: with a = (zstd/wstd)*(W-muW),
    mu[b,m] = r*d1[b] + r*s*d2[b] + s^2*d3[b],
    d1 = feature@muW, d2 = feature@a, d3 = feature@Z,
so everything is ONE fused matmul per (chunk, batch-tile):
    feature @ [ -wrho.T | muW a Z 0 | wpi.T | wvar.T ]
with s = (1+tanh(u))*exp(-u), and tau=1e-4 dropped (8e-6 of max|var|).

v3 structure (per core, BC=256 rows = 2 partition tiles of 128):
 - ONE fused DRAM block [128p, 8c, 644] = [featT0|featT1|wcat] per chunk,
   streamed as 4 pair-chunk DMAs alternating the SP and ACT HWDGE queues
   (measured fastest: fewer DMAs lose, more lose, 1 queue loses).
 - 18 matmuls total (2 bias + 16 data; measured ~90ns fixed cost per
   matmul makes instruction count matter more than the cost model says).
   No PE warmup fillers (measured: they only add time).
 - Epilogue fused over both tiles as [128, 2, .] ops; z=1-erho and
   var=z*var0 on gpsimd in parallel with the DVE pi chain; per-tile ops
   only where per-partition scalars force it (q/rq/mu, pi scale).
 - Outputs in fp16; pi|var go out via a PREPARED SWDGE writeback fired by
   a ~25ns trigger; mu via a normal HWDGE DMA.
"""

import time
from contextlib import ExitStack

import numpy as np

import concourse.bass as bass
import concourse.bacc as bacc
import concourse.mybir as mybir
import concourse.tile as tile
from concourse.bass_utils import run_bass_kernel_spmd

B, F, M = 2048, 1024, 128
NCORES = 8
BC = B // NCORES            # 256 rows/core
KC = F // 128               # 8 contraction chunks
NW = 3 * M + 4              # 388 fused psum cols: [-u | d1 d2 d3 pad | pi | var]
RHO_1 = np.float32(0.95)
S0 = float(np.sqrt(np.float32(1.0) - RHO_1 * RHO_1))

F32 = mybir.dt.float32
F16 = mybir.dt.float16
AF = mybir.ActivationFunctionType
OP = mybir.AluOpType
AX = mybir.AxisListType

MM_NP = np.float16

# psum column layout
C_U0, C_U1 = 0, M               # -u (negated rho logits)
C_D0, C_D1 = M, M + 4           # d1 d2 d3 pad
C_P0, C_P1 = M + 4, 2 * M + 4   # pi logits
C_V0, C_V1 = 2 * M + 4, 3 * M + 4  # var logits

# --- tuning flags (A/B tested on hardware; see docstring) ---
USE_TRIGGER = True        # pi|var out via prepared SWDGE writeback + trigger
VAR_ON_POOL = False       # measured: var on DVE beats gpsimd with trigger on
ACT_ORDER_Q_EARLY = True
STOP_AFTER = None         # None | "dma" | "mm" diagnostics
BIAS_Q = "gpsimd"
N_FILLERS = 0             # measured: PE warmup fillers only add time here


def _emit_body(nc, tc, pools, dram):
    consts, blkpool, psum, work = pools
    blk_d, bias_d, outmu_d, outpv_d = dram

    # bias block on the SWDGE queue (keeps the two HWDGE queues clear)
    if BIAS_Q != "none":
        bias = consts.tile([1, 128 + NW], F16, tag="bias", name="bias")
        getattr(nc, BIAS_Q).dma_start(bias[:], bias_d)

    out_pv = None
    if STOP_AFTER is None:
        out_pv = work.tile([128, 1, 2, 2 * M], F16, tag="out_pv", name="out_pv")
    if USE_TRIGGER and STOP_AFTER is None:
        # descriptors generated up front on the idle gpsimd engine; firing
        # them after the last compute costs ~25ns instead of a full HWDGE
        # gen + DGE delay (~1.3us).
        ctx0 = consts.tile([128, 2], mybir.dt.int32, tag="ctx0", name="ctx0")
        nc.vector.memset(ctx0[:], 0)
        pv_sem = nc.alloc_semaphore("pv_dma")
        nc.gpsimd.kv_writeback(outpv_d, out_pv[:], ctx0[:],
                               prepare_only=True, sem=pv_sem)

    # input stream: 4 pair-chunk DMAs alternating SP/ACT
    blk = blkpool.tile([128, KC, 2 * 128 + NW], F16, tag="blk", name="blk")
    for i in range(KC // 2):
        q = nc.sync if i % 2 == 0 else nc.scalar
        q.dma_start(blk[:, 2 * i:2 * i + 2, :], blk_d[:, 2 * i:2 * i + 2, :])

    if STOP_AFTER == "dma":
        o = work.tile([1, 16], F16, tag="tiny", name="tiny")
        nc.vector.tensor_copy(o[:], blk[0:1, KC - 1, 0:16])
        nc.sync.dma_start(outmu_d[0, 0:1, 0:16], o[:])
        return

    # matmuls: one per (chunk, tile), 388 cols each
    P = psum.tile([128, 2, 512], F32, tag="P", name="P")
    if N_FILLERS:
        wsrc = consts.tile([1, 128], F16, tag="pe_w", name="pe_w")
        nc.vector.memset(wsrc[:], 1.0)
        msrc = consts.tile([1, 512], F16, tag="pe_m", name="pe_m")
        nc.vector.memset(msrc[:], 1.0)
        scratch = psum.tile([128, 512], F32, tag="pe_scratch", name="pe_scratch")
        for _ in range(N_FILLERS):
            nc.tensor.matmul(scratch[:], wsrc[:], msrc[:], start=True, stop=True)
    for t in range(2):
        nc.tensor.matmul(P[:, t, 0:NW], bias[:, 0:128], bias[:, 128:128 + NW],
                         start=True, stop=False)
    for c in range(KC):
        for t in range(2):
            nc.tensor.matmul(P[:, t, 0:NW],
                             blk[:, c, t * 128:(t + 1) * 128],
                             blk[:, c, 256:256 + NW],
                             start=False, stop=(c == KC - 1))

    if STOP_AFTER == "mm":
        o = work.tile([1, 16], F32, tag="tiny", name="tiny")
        nc.vector.tensor_copy(o[:], P[0:1, 0, 0:16])
        o2 = work.tile([1, 16], F16, tag="tiny2", name="tiny2")
        nc.vector.tensor_copy(o2[:], o[:])
        nc.sync.dma_start(outmu_d[0, 0:1, 0:16], o2[:])
        return

    # ---- epilogue (fused [128, 2, .] over both batch tiles) ----
    r = work.tile([128, 2, M], F32, tag="r", name="r")
    nc.scalar.activation(r[:], P[:, :, C_U0:C_U1], AF.Tanh, scale=-1.0)
    eneg = work.tile([128, 2, M], F32, tag="eneg", name="eneg")
    nc.scalar.activation(eneg[:], P[:, :, C_U0:C_U1], AF.Exp)

    dsb = work.tile([128, 2, 3], F32, tag="dsb", name="dsb")
    nc.vector.tensor_copy(dsb[:], P[:, :, C_D0:C_D0 + 3])

    # erho from the UNCLAMPED r; z column 0 patched to the constant 1-e^0.95
    erho = work.tile([128, 2, M], F32, tag="erho", name="erho")
    E2 = work.tile([128, 2, 2 * M], F32, tag="E2", name="E2")
    q = work.tile([128, 2, M], F32, tag="q", name="q")
    s = work.tile([128, 2, M], F32, tag="s", name="s")

    nc.scalar.activation(erho[:], r[:], AF.Exp)
    if not ACT_ORDER_Q_EARLY:
        nc.scalar.activation(E2[:], P[:, :, C_P0:C_V1], AF.Exp)

    zeng = nc.gpsimd if VAR_ON_POOL else nc.vector
    z = work.tile([128, 2, M], F32, tag="z", name="z")
    zeng.tensor_scalar(z[:], erho[:], -1.0, 1.0, OP.mult, OP.add)
    for t in range(2):
        zeng.memset(z[:, t, 0:1], float(1.0 - np.exp(RHO_1)))
        nc.vector.memset(r[:, t, 0:1], float(RHO_1))

    nc.vector.scalar_tensor_tensor(s[:], r[:], 1.0, eneg[:], OP.add, OP.mult)
    for t in range(2):
        nc.vector.memset(s[:, t, 0:1], S0)
    ss = work.tile([128, 2, M], F32, tag="ss", name="ss")
    nc.vector.tensor_tensor(ss[:], s[:], s[:], OP.mult)

    for t in range(2):
        nc.scalar.activation(q[:, t, :], s[:, t, :], AF.Identity,
                             bias=dsb[:, t, 0:1], scale=dsb[:, t, 1:2])
    if ACT_ORDER_Q_EARLY:
        nc.scalar.activation(E2[:], P[:, :, C_P0:C_V1], AF.Exp)

    out_mu = work.tile([128, 2, M], F16, tag="out_mu", name="out_mu")
    rq = work.tile([128, 2, M], F32, tag="rq", name="rq")
    for t in range(2):
        nc.vector.tensor_tensor(rq[:, t, :], r[:, t, :], q[:, t, :], OP.mult)
        nc.vector.scalar_tensor_tensor(out_mu[:, t, :], ss[:, t, :],
                                       dsb[:, t, 2:3], rq[:, t, :],
                                       OP.mult, OP.add)
    nc.sync.dma_start(outmu_d.rearrange("t p j -> p t j"), out_mu[:])

    epi, var0 = E2[:, :, 0:M], E2[:, :, M:2 * M]
    zeng.tensor_tensor(out_pv[:, 0, :, 0:M], z[:], var0, OP.mult)
    ssum = work.tile([128, 2, 1], F32, tag="ssum", name="ssum")
    nc.vector.tensor_reduce(ssum[:], epi, AX.X, OP.add)
    rsum = work.tile([128, 2, 1], F32, tag="rsum", name="rsum")
    nc.vector.reciprocal(rsum[:], ssum[:])
    for t in range(2):
        nc.vector.tensor_scalar_mul(out_pv[:, 0, t, M:2 * M], E2[:, t, 0:M],
                                    rsum[:, t, 0:1])
    if USE_TRIGGER:
        # signals_writable puts the writer deps on the trigger itself (the
        # tile scheduler may otherwise hoist the trigger above the writers)
        nc.gpsimd.trigger_dma(count=None, signals_writable=[out_pv[:]])
    else:
        nc.sync.dma_start(outpv_d.rearrange("t p o j -> p o t j"), out_pv[:])


def _declare_io(nc):
    blk_d = nc.dram_tensor("blk", [128, KC, 2 * 128 + NW], F16,
                           kind="ExternalInput").ap()
    bias_d = nc.dram_tensor("bias", [1, 128 + NW], F16,
                            kind="ExternalInput").ap()
    outmu_d = nc.dram_tensor("out_mu", [2, 128, M], F16,
                             kind="ExternalOutput").ap()
    outpv_d = nc.dram_tensor("out_pv", [2, 128, 1, 2 * M], F16,
                             kind="ExternalOutput").ap()
    return blk_d, bias_d, outmu_d, outpv_d


def _warmup_act(nc, consts):
    # Load the Tanh/Exp ACT table immediately (one-time ~1.3us).
    warm_in = consts.tile([128, 1], F32, tag="warm_in", name="warm_in")
    warm_out = consts.tile([128, 1], F32, tag="warm_out", name="warm_out")
    nc.vector.memset(warm_in[:], 0.0)
    nc.scalar.activation(warm_out[:], warm_in[:], AF.Exp)


def _build(reps=None):
    nc = bacc.Bacc("TRN2", target_bir_lowering=False, debug=False)
    dram = _declare_io(nc)
    with tile.TileContext(nc) as tc, ExitStack() as ctx:
        consts = ctx.enter_context(tc.tile_pool(name="consts", bufs=1))
        blkpool = ctx.enter_context(tc.tile_pool(name="blk", bufs=1))
        psum = ctx.enter_context(tc.tile_pool(name="psum", bufs=1, space="PSUM"))
        work = ctx.enter_context(tc.tile_pool(name="work", bufs=1))
        pools = (consts, blkpool, psum, work)
        _warmup_act(nc, consts)
        if reps is None:
            _emit_body(nc, tc, pools, dram)
        else:
            with tc.For_i(0, reps, 1):
                _emit_body(nc, tc, pools, dram)
    nc.compile()
    return nc


def build_loop_nc(reps):
    return _build(reps=reps)


_CACHE = {}


def _get_nc():
    if "nc" not in _CACHE:
        _CACHE["nc"] = _build()
    return _CACHE["nc"]


def _host_prep(inputs):
    f32 = np.float32
    feature = np.ascontiguousarray(inputs["feature"], dtype=f32)
    muW = np.asarray(inputs["muW"], dtype=f32)
    W = np.asarray(inputs["W"], dtype=f32)
    Z = np.asarray(inputs["Z"], dtype=f32)
    logvarW = np.asarray(inputs["logvarW"], dtype=f32)
    logvarZ = np.asarray(inputs["logvarZ"], dtype=f32)

    wstd = np.sqrt(np.exp(logvarW)).astype(f32)
    zstd = np.sqrt(np.exp(logvarZ)).astype(f32)
    a = ((zstd / wstd).astype(f32) * (W - muW)).astype(f32)
    v3 = np.stack([muW, a, Z, np.zeros_like(muW)], axis=1)      # [F, 4]

    wcat = np.concatenate(
        [-np.asarray(inputs["h2rho_w"], dtype=f32).T, v3,
         np.asarray(inputs["h2pi_w"], dtype=f32).T,
         np.asarray(inputs["h2var_w"], dtype=f32).T], axis=1)    # [F, 388]
    wcat = wcat.reshape(KC, 128, NW).astype(MM_NP)

    bias = np.concatenate(
        [np.ones(128, dtype=f32),
         -np.asarray(inputs["h2rho_b"], dtype=f32), np.zeros(4, dtype=f32),
         np.asarray(inputs["h2pi_b"], dtype=f32),
         np.asarray(inputs["h2var_b"], dtype=f32)]
    ).reshape(1, 128 + NW).astype(MM_NP)

    in_maps = []
    for cidx in range(NCORES):
        shard = feature[cidx * BC:(cidx + 1) * BC]       # [256, F]
        featT = shard.T.reshape(KC, 128, BC).astype(MM_NP)
        blk = np.empty((128, KC, 2 * 128 + NW), dtype=MM_NP)
        blk[:, :, 0:256] = featT.transpose(1, 0, 2)
        blk[:, :, 256:256 + NW] = wcat.transpose(1, 0, 2)
        in_maps.append({"blk": np.ascontiguousarray(blk), "bias": bias})
    return in_maps


def kernel(**inputs):
    nc = _get_nc()
    in_maps = _host_prep(inputs)
    res = run_bass_kernel_spmd(nc, in_maps, list(range(NCORES)))
    mu = np.concatenate(
        [res.results[c]["out_mu"].reshape(BC, M) for c in range(NCORES)],
        axis=0).astype(np.float32)
    pv = np.concatenate(
        [res.results[c]["out_pv"].reshape(BC, 2 * M) for c in range(NCORES)],
        axis=0).astype(np.float32)
    var = np.ascontiguousarray(pv[:, 0:M])
    pi = np.ascontiguousarray(pv[:, M:2 * M])
    return pi, mu, var
